# revision 4
# baseline (speedup 1.0000x reference)
"""Trainium2 Bass kernel: bidirectional self-attention with interleaved RoPE.

Problem (full shapes): x [4, 2048, 2048] f32, w_qkv [2048, 6144], w_proj
[2048, 2048].  y = SDPA(rope(q), rope(k), v) @ w_proj with 16 heads, hd=128.

Sharding: batch x head-group hybrid over 8 cores.  Core c handles batch
b = c//2 and head group g = c%2 (8 of the 16 heads).  Each core computes a
partial projection output [T, C] (its heads' contribution); the host sums
the two partials per batch (the w_proj row-parallel all-reduce done on host).

Device kernel (per core): one software-pipelined pass.  Everything in
transposed activation layout so no on-chip transposes are ever needed:
  xT [C, T]                  (host-transposed input slice, bf16)
  v   = xT-tiles^T @ wv      [t, d] natural layout, all heads upfront
                             (wv is staged through the not-yet-used y tiles)
  qT/kT = W^T xT             [hd, T] per head, heads 0+1 projected in the
                             prologue (doubles the PE work unlocked per
                             arriving xt tile in the DMA-paced start),
                             then head h+2 streamed as filler during head h
  rope:  qT_rope = qT*cosT + shuffle(qT)*sinT   (DVE; sign folded in sinT)
  ST    = kT-tiles vs qT_rope                   -> S^T [k, q] tiles in psum
  E     = exp(ST * 1/sqrt(hd))                  (ACT, no max-subtraction:
                                                scores are O(5) for randn)
  denom = DVE running sum over E k-tiles, then one ones-matmul
  yT    = V-contraction (lhsT = v_nat [k, d], rhs = E [k, q]) -> [d, q]
  y_sb  = yT * reciprocal(denom)
  out   = y_sb^T-tiles @ w_proj-rows            -> partial [T, C] bf16
Attention iterations (head, q-half) software-pipeline on the Tile priority
scheduler: ST(kt)->exp(kt) on ACT, PV(kt) consumes exp output in lockstep,
and the qk-filler matmuls keep the tensor engine fed while ACT chews
through the exps.

All DRAM tensors are host-repacked partition-major so DMAs move contiguous
runs >= 2KB per partition row where granularity allows (the 16 shared DMA
engines are run-length bound early: 1KB runs deliver ~17 B/ns/engine vs
~24 at 2KB).  xt chunks 0/1 stay at [128,512] (1KB runs) because the start
is paced by per-f-tile arrival, not bandwidth.  A short burst of warmup
matmuls on a memset scratch tile keeps the PE busy (and HAM-warm) through
the DMA preamble before real operands land.
"""

import math
import os

import numpy as np

N_HEAD = 16
ROPE_BASE = 10000.0
HD = 128          # head dim == partition count; the kernel relies on this
PP = 128          # partitions

# full-problem constants (hardcoded per contract; kernel.py reads no files)
FULL_B, FULL_T, FULL_C = 4, 2048, 2048
N_CORES = 8

_NC_CACHE = {}


# ----------------------------------------------------------------- host math

def _rope_tables(T, hd=HD):
    """cos/sin tables, transposed to [hd, T] (lucidrains interleaved style)."""
    inv_freq = 1.0 / (ROPE_BASE ** (np.arange(0, hd, 2, dtype=np.float64) / hd))
    ang = np.arange(T, dtype=np.float64)[:, None] * inv_freq[None, :]
    ang = np.repeat(ang, 2, axis=1)                       # [T, hd]
    return np.cos(ang).T.copy(), np.sin(ang).T.copy()     # [hd, T]


def _pmajor(a, nf):
    """[nf*128, W] -> [128, nf*W]: partition-major repack so partition p's
    SBUF row is one contiguous DRAM run (f-tiles side by side)."""
    r, w = a.shape
    assert r == nf * PP
    return np.ascontiguousarray(
        a.reshape(nf, PP, w).transpose(1, 0, 2).reshape(PP, nf * w))


# ------------------------------------------------------------ device builder

def build_nc(T, F, HL, CO, compile_now=True):
    """Build (and compile) the per-core Bass program.

    T: sequence length, F: model/contraction dim, HL: local heads,
    CO: output width.  hd is fixed at 128."""
    from contextlib import ExitStack

    import concourse.tile as tile
    from concourse import bacc, mybir
    from concourse.bass import ds, ts

    hd = HD
    CL = HL * hd                       # local v / proj-row width
    NT, NF = T // PP, F // PP          # k-tiles / contraction tiles
    SC = 512                           # single-matmul moving width
    NJ = T // SC                       # xt chunks along t
    TPJ = SC // PP                     # t-tiles per xt chunk
    C2 = 2 * SC                        # attention q-chunk (2 psum banks)
    NC2 = T // C2                      # q-halves per head
    NCS = CL // PP                     # proj contraction tiles (= HL)
    NOC = CO // SC                     # proj output chunks
    NWU = 14                           # PE warmup matmuls bridging the preamble
    scale = 1.0 / math.sqrt(hd)
    bf = mybir.dt.bfloat16
    f32 = mybir.dt.float32

    nc = bacc.Bacc(
        "TRN2",
        target_bir_lowering=False,
        debug=False,
        enable_asserts=False,
        num_devices=1,
    )

    # all inputs partition-major: row p holds every f-tile's row p contiguously
    xt_d = nc.declare_dram_parameter("xt", [PP, NF * T], bf, isOutput=False)
    wqk_d = nc.declare_dram_parameter("wqk", [PP, HL * NF * 2 * PP], bf,
                                      isOutput=False)
    wv_d = nc.declare_dram_parameter("wv", [PP, NF * CL], bf, isOutput=False)
    wp_d = nc.declare_dram_parameter("wp", [PP, NCS * CO], bf, isOutput=False)
    cost_d = nc.declare_dram_parameter("cost", [PP, T], bf, isOutput=False)
    sint_d = nc.declare_dram_parameter("sint", [PP, T], bf, isOutput=False)
    out_d = nc.declare_dram_parameter("out", [T, CO], bf, isOutput=True)

    xt_r = xt_d.ap().rearrange("p (nf t) -> nf p t", nf=NF)
    wqk_r = wqk_d.ap().rearrange("p (h c) -> h p c", h=HL)
    wv_r = wv_d.ap()                   # [128, NF*CL]; y-tile i <- cols i*2CL
    wp_r = wp_d.ap().rearrange("p (ncs c) -> ncs p c", ncs=NCS)
    out_r = out_d.ap().rearrange("(nt p) c -> nt p c", p=PP)

    with tile.TileContext(nc) as tc, ExitStack() as octx:

        # ------------- long-lived pools (strict stack: opened first) -------
        # one pool per bufs-class keeps the end-of-program semaphore flush
        # short; tags still separate the rings
        st_pool = octx.enter_context(tc.tile_pool(name="st", bufs=1))
        v_sb = [st_pool.tile([PP, CL], bf, tag=f"v{t}", name=f"v{t}")
                for t in range(NT)]
        # index (h%2)*2 + {0:q, 1:k}
        qk_sb = [st_pool.tile([PP, T], bf, tag=f"qk{i}", name=f"qk{i}")
                 for i in range(4)]
        cost_sb = st_pool.tile([PP, T], bf, tag="cost")
        sint_sb = st_pool.tile([PP, T], bf, tag="sint")
        ones_sb = st_pool.tile([PP, PP], bf, tag="ones")
        # packed per-head-slot qk weights: 4 tiles x 4 f-blocks of [q|k]
        wq_sb = [[st_pool.tile([PP, 4 * 2 * PP], bf, tag=f"wq{b}_{g}",
                               name=f"wq{b}_{g}") for g in range(NF // 4)]
                 for b in range(2)]
        wpre_sb = [st_pool.tile([PP, CO], bf, tag=f"wp0_{i}", name=f"wp0_{i}")
                   for i in range(3)]
        y_sb = [st_pool.tile([PP, T], bf, tag=f"y{h}", name=f"y{h}")
                for h in range(HL)]
        vw_pool = octx.enter_context(tc.tile_pool(name="vw", bufs=2))
        e_pool = octx.enter_context(tc.tile_pool(name="e", bufs=7))
        pq_pool = octx.enter_context(
            tc.tile_pool(name="pq", bufs=2, space="PSUM"))
        pst_pool = octx.enter_context(
            tc.tile_pool(name="pst", bufs=2, space="PSUM"))
        py_pool = octx.enter_context(
            tc.tile_pool(name="py", bufs=2, space="PSUM"))

        # xt on its own stack so it can be released before the wp tiles open
        xt_stack = ExitStack()
        xt_pool = xt_stack.enter_context(tc.tile_pool(name="xt", bufs=1))
        # chunks 0,1 fine-grained ([128,512]: arrival paces the start),
        # chunks 2,3 in one [128,1024] tile (2KB DMA runs)
        xt_sb = [[xt_pool.tile([PP, SC], bf, tag=f"xt{f}_0", name=f"xt{f}_0"),
                  xt_pool.tile([PP, SC], bf, tag=f"xt{f}_1", name=f"xt{f}_1"),
                  xt_pool.tile([PP, 2 * SC], bf, tag=f"xt{f}_23",
                               name=f"xt{f}_23")]
                 for f in range(NF)]

        def xt_ap(f, j):               # [PP, SC] slice of chunk j
            if j < 2:
                return xt_sb[f][j][:]
            return xt_sb[f][2][:, ds((j - 2) * SC, SC)]

        def xt_tt(f, t):               # [PP, PP] t-tile (t in 0..NT-1)
            if t < 2 * TPJ:
                return xt_sb[f][t // TPJ][:, ts(t % TPJ, PP)]
            return xt_sb[f][2][:, ts(t - 2 * TPJ, PP)]

        def wq_ap(b, f, m):            # [PP, PP] lhsT for head-slot b
            return wq_sb[b][f // 4][:, ds((f % 4) * 2 * PP + m * PP, PP)]

        # ---------------- PE warmup: keep HAM warm through the preamble -----
        # memset scratch (v_sb[15] is not written until the last v tile's
        # copies, long after these reads), then dep-free matmuls so the PE
        # is busy from iram-load time instead of first-DMA-landing time.
        nc.vector.memset(v_sb[NT - 1][:, ds(0, SC)], 0)
        nc.vector.memset(ones_sb[:], 1.0)
        for i in range(NWU):
            pwu = pq_pool.tile([PP, SC], f32, tag="pq", name="pq")
            nc.tensor.matmul(
                pwu[:],
                lhsT=v_sb[NT - 1][:, ts(0, PP)],
                rhs=v_sb[NT - 1][:, ds(0, SC)],
                start=True,
                stop=True,
            )

        # ---------------- helper emitters ----------------------------------
        def emit_qk_chunk(h, ci):
            """One [hd, SC] chunk of head h's kT (ci 0..3) or qT (ci 4..7):
            16-matmul F-contraction, psum->sbuf copy + rope (DVE)."""
            b = h % 2
            m = 1 - ci // NJ           # 0..3 -> k (m=1), 4..7 -> q (m=0)
            j = ci % NJ
            pqt = pq_pool.tile([PP, SC], f32, tag="pq")
            for f in range(NF):
                nc.tensor.matmul(
                    pqt[:],
                    lhsT=wq_ap(b, f, m),
                    rhs=xt_ap(f, j),
                    start=(f == 0),
                    stop=(f == NF - 1),
                )
            qsb = vw_pool.tile([PP, SC], bf, tag="qsb")
            # DVE, not ACT: keeps the scalar engine pure-exp in the heads
            # phase (exp paces the PV lockstep; a copy in front of the next
            # iteration's first exp delays the whole chain).
            nc.vector.tensor_copy(qsb[:], pqt[:])
            # rotate_half = pair-swap of partitions (same permutation in every
            # 32-partition quadrant); the +-1 sign is folded into sint host-side
            qrot = vw_pool.tile([PP, SC], bf, tag="qrot")
            nc.vector.stream_shuffle(qrot[:], qsb[:], [i ^ 1 for i in range(32)])
            nc.vector.tensor_mul(qsb[:], qsb[:], cost_sb[:, ds(j * SC, SC)])
            nc.vector.tensor_mul(qrot[:], qrot[:], sint_sb[:, ds(j * SC, SC)])
            nc.vector.tensor_add(
                qk_sb[2 * (h % 2) + m][:, ds(j * SC, SC)], qsb[:], qrot[:])

        def psum_rr(i, shape):
            """Round-robin a [PP, SC] psum tile across the three psum pools."""
            pool, tag = ((pst_pool, "pst"), (py_pool, "py"), (pq_pool, "pq"))[i]
            return pool.tile(shape, f32, tag=tag, name=tag)

        def emit_proj_unit(t, ocp, rr0, rr1, on_act=False):
            """One [t-tile, 2*SC] unit of the output projection + one store.

            Two psum chains (adjacent SC columns), two casts to one bf16
            staging tile (split ACT/DVE so neither engine eats both), one
            2KB-run DMA."""
            ost = ost_pool.tile([PP, 2 * SC], bf, tag="ost")
            for half, rr in ((0, rr0), (1, rr1)):
                po = psum_rr(rr, [PP, SC])
                for cs in range(NCS):
                    nc.tensor.matmul(
                        po[:],
                        lhsT=y_sb[cs][:, ts(t, PP)],
                        rhs=wp_sb[cs][:, ds(ocp * 2 * SC + half * SC, SC)],
                        start=(cs == 0),
                        stop=(cs == NCS - 1),
                    )
                if (half == 0) == on_act:
                    nc.scalar.copy(ost[:, ds(half * SC, SC)], po[:])
                else:
                    nc.vector.tensor_copy(ost[:, ds(half * SC, SC)], po[:])
            # out-DMA alternates queues so neither HWDGE ring backs up
            if (t + ocp) % 2 == 0:
                nc.sync.dma_start(out_r[t][:, ds(ocp * 2 * SC, 2 * SC)], ost[:])
            else:
                nc.scalar.dma_start(out_r[t][:, ds(ocp * 2 * SC, 2 * SC)], ost[:])

        # ---------------- prologue: DMAs, head-0/1 qk, v GEMM ---------------
        # wq0 first (striped across all three queues), rope tables, then the
        # xt chunk-0/1 tiles f-striped so the qk chains advance at aggregate
        # arrival rate, wq1, wv, xt chunks 2/3.
        qs = (nc.sync, nc.scalar, nc.gpsimd)
        for g in range(NF // 4):
            qs[g % 3].dma_start(wq_sb[0][g][:], wqk_r[0][:, ds(g * 8 * PP,
                                                               8 * PP)])
        nc.scalar.dma_start(cost_sb[:], cost_d.ap())
        nc.scalar.dma_start(sint_sb[:], sint_d.ap())
        for j in range(2):
            for f in range(NF):
                qs[(f + j) % 3].dma_start(xt_sb[f][j][:],
                                          xt_r[f][:, ds(j * SC, SC)])
        for g in range(NF // 4):
            nc.gpsimd.dma_start(wq_sb[1][g][:], wqk_r[1][:, ds(g * 8 * PP,
                                                               8 * PP)])
        for i in range(NCS):           # wv: y-tile i <- packed cols [2i*CL]
            qs[i % 3].dma_start(y_sb[i][:], wv_r[:, ds(i * 2 * CL, 2 * CL)])
        for f in range(NF):
            qs[f % 3].dma_start(xt_sb[f][2][:], xt_r[f][:, ds(2 * SC, 2 * SC)])
        # (wp row-tiles 0-2 are preloaded later, at head 2: they are not
        # needed until head 7, and their 1.5MB would steal HBM bandwidth
        # from the xt/wv set that paces the DMA-starved prologue)

        def wv_ap(f, c):               # wv f-tile chunk c staged in y space
            return y_sb[f // 2][:, ds((f % 2) * CL + c * SC, SC)]

        def emit_v_tile(t):
            for c in range(CL // SC):
                ps = psum_rr(c % 2, [PP, SC])
                for f in range(NF):
                    nc.tensor.matmul(
                        ps[:],
                        lhsT=xt_tt(f, t),
                        rhs=wv_ap(f, c),
                        start=(f == 0),
                        stop=(f == NF - 1),
                    )
                nc.vector.tensor_copy(v_sb[t][:, ts(c, SC)], ps[:])

        # heads 0 AND 1 projected here: each arriving xt f-tile unlocks four
        # matmuls (k+q for two heads) instead of two, which is what keeps the
        # PE fed while the DMA engines stream the first megabytes in
        for j in range(NJ):
            for h01 in range(2):
                emit_qk_chunk(h01, j)           # k chunk j
                emit_qk_chunk(h01, NJ + j)      # q chunk j
        for t in range(NT):
            emit_v_tile(t)

        # ---------------- heads loop: attention + pipelined qk filler -------
        for h in range(HL):
            if h + 2 < HL:
                # wq for head h+2 into slot h%2 (read by qk_h, already done)
                b = (h + 2) % 2
                for g in range(NF // 4):
                    nc.gpsimd.dma_start(wq_sb[b][g][:],
                                        wqk_r[h + 2][:, ds(g * 8 * PP, 8 * PP)])
            if h == 2:
                for i in range(3):
                    nc.gpsimd.dma_start(wpre_sb[i][:], wp_r[i])
            if h == HL - 2:
                # xt's last readers (head-7 qk, emitted at h5) finish during
                # this head: close the pool now so the wp DMAs (WAR-gated on
                # those readers) land during h6, not h7
                xt_stack.close()
                wp_pool = octx.enter_context(tc.tile_pool(name="wp", bufs=1))
                wp_sb = wpre_sb + [
                    wp_pool.tile([PP, CO], bf, tag=f"wp{cs}", name=f"wp{cs}")
                    for cs in range(3, NCS)]
                # column-halves: the split chains and early proj units only
                # touch wp columns 0..1023, so land those first
                for cs in range(3, NCS):
                    qs[cs % 3].dma_start(
                        wp_sb[cs][:, ds(0, C2)], wp_r[cs][:, ds(0, C2)])
                for cs in range(3, NCS):
                    qs[cs % 3].dma_start(
                        wp_sb[cs][:, ds(C2, C2)], wp_r[cs][:, ds(C2, C2)])
                ost_pool = octx.enter_context(tc.tile_pool(name="ost", bufs=8))

            q_sb = qk_sb[2 * (h % 2)]
            k_sb = qk_sb[2 * (h % 2) + 1]
            for c2 in range(NC2):
                if h == HL - 1 and c2 == 0:
                    # split proj chains emitted FIRST: ready the moment head 7
                    # starts (wp0 preloaded, y0..y6 done) -- they fill the
                    # head-6-to-7 pipeline drain, when no qk filler exists
                    pend = []
                    for oc in (0, 1):
                        po = pq_pool.tile([PP, SC], f32, tag="pq", name="pq")
                        for cs in range(NCS - 1):
                            nc.tensor.matmul(
                                po[:],
                                lhsT=y_sb[cs][:, ts(0, PP)],
                                rhs=wp_sb[cs][:, ds(oc * SC, SC)],
                                start=(cs == 0),
                                stop=False,
                            )
                        pend.append((oc, po))
                # ST + exp + denominator running sum, kt-streamed
                es = []
                stot = None
                for kt in range(NT):
                    pst = pst_pool.tile([PP, C2], f32, tag="pst", name="pst")
                    for s in range(2):
                        nc.tensor.matmul(
                            pst[:, ts(s, SC)],
                            lhsT=k_sb[:, ts(kt, PP)],
                            rhs=q_sb[:, ds(c2 * C2 + s * SC, SC)],
                            start=True,
                            stop=True,
                        )
                    e = e_pool.tile([PP, C2], bf, tag="e", name="e")
                    nc.scalar.activation(
                        e[:], pst[:],
                        mybir.ActivationFunctionType.Exp,
                        bias=0.0, scale=scale,
                    )
                    es.append(e)
                    # running softmax-denominator sum: keeps the post-last-exp
                    # serial DVE tail to a single add
                    if kt == 1:
                        stot = vw_pool.tile([PP, C2], bf, tag="stot")
                        nc.vector.tensor_add(stot[:], es[0][:], es[1][:])
                    elif kt > 1:
                        nc.vector.tensor_add(stot[:], stot[:], es[kt][:])
                # PV: contract all k-tiles into y^T psum
                py_s = [py_pool.tile([PP, SC], f32, tag="py", name="py")
                        for s in range(2)]
                for kt in range(NT):
                    for s in range(2):
                        nc.tensor.matmul(
                            py_s[s][:],
                            lhsT=v_sb[kt][:, ts(h, PP)],
                            rhs=es[kt][:, ts(s, SC)],
                            start=(kt == 0),
                            stop=(kt == NT - 1),
                        )
                # denominator matmul + normalization: emitted BEFORE the
                # filler so the norm (which releases the py slots gating the
                # next iteration's PV) outranks the rope work
                pden = pst_pool.tile([PP, C2], f32, tag="pst", name="pst")
                for s in range(2):
                    nc.tensor.matmul(
                        pden[:, ts(s, SC)],
                        lhsT=ones_sb[:],
                        rhs=stot[:, ts(s, SC)],
                        start=True,
                        stop=True,
                    )
                for s in range(2):
                    inv = vw_pool.tile([PP, SC], f32, tag="inv", name="inv")
                    nc.vector.reciprocal_approx_fast(inv[:], pden[:, ts(s, SC)])
                    nc.vector.tensor_mul(
                        y_sb[h][:, ds(c2 * C2 + s * SC, SC)],
                        py_s[s][:], inv[:])
                # lower-priority PE filler: head h+2's qk GEMM, or early proj.
                # h+2 shares the qk slot head h is READING, so emission order
                # must put every slot-write after the last emitted ST that
                # reads that region: q cols 0:1024 are only read by c2=0's ST
                # (safe to emit in c2=0's filler), but k_sb is read by every
                # ST, so all k chunks (and q cols 1024:) go after c2=1's STs.
                # (The 16 matmuls of a chunk have no dependency on the qk
                # slot -- only the final DVE add does -- so the PE work flows
                # even while the add's WAR on head h's ST reads resolves.)
                if h + 2 < HL:
                    for ci in ((NJ, NJ + 1) if c2 == 0
                               else (NJ + 2, NJ + 3, 0, 1, 2, 3)):
                        emit_qk_chunk(h + 2, ci)
                elif h == HL - 1 and c2 == NC2 - 1:
                    ost = ost_pool.tile([PP, 2 * SC], bf, tag="ost")
                    for oc, po in pend:
                        nc.tensor.matmul(
                            po[:],
                            lhsT=y_sb[NCS - 1][:, ts(0, PP)],
                            rhs=wp_sb[NCS - 1][:, ds(oc * SC, SC)],
                            start=False,
                            stop=True,
                        )
                        nc.vector.tensor_copy(ost[:, ds(oc * SC, SC)], po[:])
                    nc.sync.dma_start(out_r[0][:, ds(0, 2 * SC)], ost[:])
                    for k, t in enumerate((1, 2, 3)):
                        emit_proj_unit(t, 0, 2, k % 2, on_act=(k % 2 == 1))

        # ---------------- tail: remaining output projection ------------------
        # units whose y7 columns landed with h7's first q-half (t < 8) go
        # first, on the uncontended pq slots, so they can fill h7-c2=1 stalls
        done = {(t, 0) for t in range(4)}
        early = [(t, ocp) for t in range(8) for ocp in range(NOC // 2)
                 if (t, ocp) not in done]
        late = [(t, ocp) for t in range(8, NT) for ocp in range(NOC // 2)]
        for i, (t, ocp) in enumerate(early + late):
            if i < 4:
                rr0, rr1 = 2, i % 2
            else:
                rr0, rr1 = i % 3, (i + 1) % 3
            emit_proj_unit(t, ocp, rr0, rr1, on_act=(i % 2 == 1 and i < 12))

    if compile_now:
        nc.compile()
    return nc


# ------------------------------------------------------------- host wrapper

def _percore_inputs(x, w_qkv, w_proj, core, HL=8):
    """Build the in_map for one core: batch b = core//2, head group g = core%2."""
    import ml_dtypes

    bf16 = ml_dtypes.bfloat16
    B, T, C = x.shape
    hd = HD
    NF = C // PP
    CL = HL * hd
    b, g = core // 2, core % 2

    cosT, sinT = _rope_tables(T)
    sign = np.where(np.arange(HD) % 2 == 0, -1.0, 1.0)[:, None]
    # per-head packed q|k weight columns, f-tile-blocked partition-major:
    # head m block = [f0: q128|k128, f1: q128|k128, ...] per partition row
    wqk = np.empty((C, HL * 2 * hd), np.float32)
    for m in range(HL):
        gm = g * HL + m
        wqk[:, m * 2 * hd: m * 2 * hd + hd] = w_qkv[:, gm * hd:(gm + 1) * hd]
        wqk[:, m * 2 * hd + hd:(m + 1) * 2 * hd] = \
            w_qkv[:, C + gm * hd: C + (gm + 1) * hd]
    wqk_p = np.concatenate(
        [_pmajor(wqk[:, m * 2 * hd:(m + 1) * 2 * hd], NF) for m in range(HL)],
        axis=1)
    vc0 = 2 * C + g * CL
    m = {
        "xt": _pmajor(np.ascontiguousarray(x[b].T), NF).astype(bf16),
        "wqk": wqk_p.astype(bf16),
        "wv": _pmajor(np.ascontiguousarray(w_qkv[:, vc0:vc0 + CL]), NF).astype(bf16),
        "wp": _pmajor(np.ascontiguousarray(w_proj[g * CL:(g + 1) * CL, :]),
                      HL).astype(bf16),
        "cost": cosT.astype(bf16),
        "sint": (sinT * sign).astype(bf16),
    }
    return m


def kernel(x, w_qkv, w_proj):
    from concourse.bass_utils import run_bass_kernel_spmd

    x = np.asarray(x, dtype=np.float32)
    w_qkv = np.asarray(w_qkv, dtype=np.float32)
    w_proj = np.asarray(w_proj, dtype=np.float32)
    B, T, C = x.shape
    HL = N_HEAD // (N_CORES // B)

    key = (T, C, HL, C)
    if key not in _NC_CACHE:
        _NC_CACHE[key] = build_nc(T, C, HL, C)
    nc = _NC_CACHE[key]

    in_maps = [_percore_inputs(x, w_qkv, w_proj, c, HL) for c in range(N_CORES)]
    trace = bool(int(os.environ.get("KERNEL_TRACE", "0")))
    res = run_bass_kernel_spmd(
        nc, in_maps, core_ids=list(range(N_CORES)), trace=trace)
    if trace:
        global LAST_EXEC_TIME_NS, LAST_RESULT
        LAST_EXEC_TIME_NS = res.exec_time_ns
        LAST_RESULT = res

    out = np.empty((B, T, C), np.float32)
    for b in range(B):
        out[b] = (res.results[2 * b]["out"].astype(np.float32)
                  + res.results[2 * b + 1]["out"].astype(np.float32))
    return out


LAST_EXEC_TIME_NS = None
LAST_RESULT = None


# revision 15
# speedup vs baseline: 1.0038x; 1.0038x over previous
"""Trainium2 Bass kernel: bidirectional self-attention with interleaved RoPE.

Problem (full shapes): x [4, 2048, 2048] f32, w_qkv [2048, 6144], w_proj
[2048, 2048].  y = SDPA(rope(q), rope(k), v) @ w_proj with 16 heads, hd=128.

Sharding: batch x head-group hybrid over 8 cores.  Core c handles batch
b = c//2 and head group g = c%2 (8 of the 16 heads).  Each core computes a
partial projection output [T, C] (its heads' contribution); the host sums
the two partials per batch (the w_proj row-parallel all-reduce done on host).

Device kernel (per core): one software-pipelined pass.  Everything in
transposed activation layout so no on-chip transposes are ever needed:
  xT [C, T]                  (host-transposed input slice, bf16)
  v   = xT-tiles^T @ wv      [t, d] natural layout, all heads upfront
                             (wv is staged through the not-yet-used y tiles)
  qT/kT = W^T xT             [hd, T] per head, heads 0+1 projected in the
                             prologue (doubles the PE work unlocked per
                             arriving xt tile in the DMA-paced start),
                             then head h+2 streamed as filler during head h
  rope:  qT_rope = qT*cosT + shuffle(qT)*sinT   (DVE; sign folded in sinT)
  ST    = kT-tiles vs qT_rope                   -> S^T [k, q] tiles in psum
  E     = exp(ST * 1/sqrt(hd))                  (ACT, no max-subtraction:
                                                scores are O(5) for randn)
  denom = DVE running sum over E k-tiles, then one ones-matmul
  yT    = V-contraction (lhsT = v_nat [k, d], rhs = E [k, q]) -> [d, q]
  y_sb  = yT * reciprocal(denom)
  out   = y_sb^T-tiles @ w_proj-rows            -> partial [T, C] bf16
Attention iterations (head, q-half) software-pipeline on the Tile priority
scheduler: ST(kt)->exp(kt) on ACT, PV(kt) consumes exp output in lockstep,
and the qk-filler matmuls keep the tensor engine fed while ACT chews
through the exps.

All DRAM tensors are host-repacked partition-major so DMAs move contiguous
runs >= 2KB per partition row where granularity allows (the 16 shared DMA
engines are run-length bound early: 1KB runs deliver ~17 B/ns/engine vs
~24 at 2KB).  xt chunks 0/1 stay at [128,512] (1KB runs) because the start
is paced by per-f-tile arrival, not bandwidth.  A short burst of warmup
matmuls on a memset scratch tile keeps the PE busy (and HAM-warm) through
the DMA preamble before real operands land.
"""

import math
import os

import numpy as np

N_HEAD = 16
ROPE_BASE = 10000.0
HD = 128          # head dim == partition count; the kernel relies on this
PP = 128          # partitions

# full-problem constants (hardcoded per contract; kernel.py reads no files)
FULL_B, FULL_T, FULL_C = 4, 2048, 2048
N_CORES = 8

_NC_CACHE = {}


# ----------------------------------------------------------------- host math

def _rope_tables(T, hd=HD):
    """cos/sin tables, transposed to [hd, T] (lucidrains interleaved style)."""
    inv_freq = 1.0 / (ROPE_BASE ** (np.arange(0, hd, 2, dtype=np.float64) / hd))
    ang = np.arange(T, dtype=np.float64)[:, None] * inv_freq[None, :]
    ang = np.repeat(ang, 2, axis=1)                       # [T, hd]
    return np.cos(ang).T.copy(), np.sin(ang).T.copy()     # [hd, T]


def _pmajor(a, nf):
    """[nf*128, W] -> [128, nf*W]: partition-major repack so partition p's
    SBUF row is one contiguous DRAM run (f-tiles side by side)."""
    r, w = a.shape
    assert r == nf * PP
    return np.ascontiguousarray(
        a.reshape(nf, PP, w).transpose(1, 0, 2).reshape(PP, nf * w))


# ------------------------------------------------------------ device builder

def build_nc(T, F, HL, CO, compile_now=True):
    """Build (and compile) the per-core Bass program.

    T: sequence length, F: model/contraction dim, HL: local heads,
    CO: output width.  hd is fixed at 128."""
    from contextlib import ExitStack

    import concourse.tile as tile
    from concourse import bacc, mybir
    from concourse.bass import ds, ts

    hd = HD
    CL = HL * hd                       # local v / proj-row width
    NT, NF = T // PP, F // PP          # k-tiles / contraction tiles
    SC = 512                           # single-matmul moving width
    NJ = T // SC                       # xt chunks along t
    TPJ = SC // PP                     # t-tiles per xt chunk
    C2 = 2 * SC                        # attention q-chunk (2 psum banks)
    NC2 = T // C2                      # q-halves per head
    NCS = CL // PP                     # proj contraction tiles (= HL)
    NOC = CO // SC                     # proj output chunks
    NWU = 8                            # PE warmup matmuls bridging the preamble
    scale = 1.0 / math.sqrt(hd)
    bf = mybir.dt.bfloat16
    f32 = mybir.dt.float32

    nc = bacc.Bacc(
        "TRN2",
        target_bir_lowering=False,
        debug=False,
        enable_asserts=False,
        num_devices=1,
    )

    # all inputs partition-major: row p holds every f-tile's row p contiguously
    xt_d = nc.declare_dram_parameter("xt", [PP, NF * T], bf, isOutput=False)
    wqk_d = nc.declare_dram_parameter("wqk", [PP, HL * NF * 2 * PP], bf,
                                      isOutput=False)
    wv_d = nc.declare_dram_parameter("wv", [PP, NF * CL], bf, isOutput=False)
    wp_d = nc.declare_dram_parameter("wp", [PP, NCS * CO], bf, isOutput=False)
    cost_d = nc.declare_dram_parameter("cost", [PP, T], bf, isOutput=False)
    sint_d = nc.declare_dram_parameter("sint", [PP, T], bf, isOutput=False)
    out_d = nc.declare_dram_parameter("out", [T, CO], bf, isOutput=True)

    xt_r = xt_d.ap().rearrange("p (nf t) -> nf p t", nf=NF)
    wqk_r = wqk_d.ap().rearrange("p (h c) -> h p c", h=HL)
    wv_r = wv_d.ap()                   # [128, NF*CL]; y-tile i <- cols i*2CL
    wp_r = wp_d.ap().rearrange("p (ncs c) -> ncs p c", ncs=NCS)
    out_r = out_d.ap().rearrange("(nt p) c -> nt p c", p=PP)

    with tile.TileContext(nc) as tc, ExitStack() as octx:

        # ------------- long-lived pools (strict stack: opened first) -------
        # one pool per bufs-class keeps the end-of-program semaphore flush
        # short; tags still separate the rings
        st_pool = octx.enter_context(tc.tile_pool(name="st", bufs=1))
        v_sb = [st_pool.tile([PP, CL], bf, tag=f"v{t}", name=f"v{t}")
                for t in range(NT)]
        # index (h%2)*2 + {0:q, 1:k}
        qk_sb = [st_pool.tile([PP, T], bf, tag=f"qk{i}", name=f"qk{i}")
                 for i in range(4)]
        cost_sb = st_pool.tile([PP, T], bf, tag="cost")
        sint_sb = st_pool.tile([PP, T], bf, tag="sint")
        ones_sb = st_pool.tile([PP, PP], bf, tag="ones")
        # packed per-head-slot qk weights: 4 tiles x 4 f-blocks of [q|k]
        wq_sb = [[st_pool.tile([PP, 4 * 2 * PP], bf, tag=f"wq{b}_{g}",
                               name=f"wq{b}_{g}") for g in range(NF // 4)]
                 for b in range(2)]
        wpre_sb = [st_pool.tile([PP, CO], bf, tag=f"wp0_{i}", name=f"wp0_{i}")
                   for i in range(3)]
        y_sb = [st_pool.tile([PP, T], bf, tag=f"y{h}", name=f"y{h}")
                for h in range(HL)]
        vw_pool = octx.enter_context(tc.tile_pool(name="vw", bufs=2))
        vr_pool = octx.enter_context(tc.tile_pool(name="vr", bufs=1))
        e_pool = octx.enter_context(tc.tile_pool(name="e", bufs=8))
        pq_pool = octx.enter_context(
            tc.tile_pool(name="pq", bufs=2, space="PSUM"))
        pst_pool = octx.enter_context(
            tc.tile_pool(name="pst", bufs=2, space="PSUM"))
        py_pool = octx.enter_context(
            tc.tile_pool(name="py", bufs=2, space="PSUM"))

        # xt on its own stack so it can be released before the wp tiles open
        xt_stack = ExitStack()
        xt_pool = xt_stack.enter_context(tc.tile_pool(name="xt", bufs=1))
        # chunks 0,1 fine-grained ([128,512]: arrival paces the start),
        # chunks 2,3 in one [128,1024] tile (2KB DMA runs)
        xt_sb = [[xt_pool.tile([PP, SC], bf, tag=f"xt{f}_0", name=f"xt{f}_0"),
                  xt_pool.tile([PP, SC], bf, tag=f"xt{f}_1", name=f"xt{f}_1"),
                  xt_pool.tile([PP, 2 * SC], bf, tag=f"xt{f}_23",
                               name=f"xt{f}_23")]
                 for f in range(NF)]

        def xt_ap(f, j):               # [PP, SC] slice of chunk j
            if j < 2:
                return xt_sb[f][j][:]
            return xt_sb[f][2][:, ds((j - 2) * SC, SC)]

        def xt_tt(f, t):               # [PP, PP] t-tile (t in 0..NT-1)
            if t < 2 * TPJ:
                return xt_sb[f][t // TPJ][:, ts(t % TPJ, PP)]
            return xt_sb[f][2][:, ts(t - 2 * TPJ, PP)]

        def wq_ap(b, f, m):            # [PP, PP] lhsT for head-slot b
            return wq_sb[b][f // 4][:, ds((f % 4) * 2 * PP + m * PP, PP)]

        # ---------------- PE warmup: keep HAM warm through the preamble -----
        # memset scratch (v_sb[15] is not written until the last v tile's
        # copies, long after these reads), then dep-free matmuls so the PE
        # is busy from iram-load time instead of first-DMA-landing time.
        nc.vector.memset(v_sb[NT - 1][:, ds(0, SC)], 0)
        nc.vector.memset(ones_sb[:], 1.0)
        for i in range(NWU):
            pwu = pq_pool.tile([PP, SC], f32, tag="pq", name="pq")
            nc.tensor.matmul(
                pwu[:],
                lhsT=v_sb[NT - 1][:, ts(0, PP)],
                rhs=v_sb[NT - 1][:, ds(0, SC)],
                start=True,
                stop=True,
            )

        # ---------------- helper emitters ----------------------------------
        def emit_qk_chunk(h, ci, rr=None):
            """One [hd, SC] chunk of head h's kT (ci 0..3) or qT (ci 4..7):
            16-matmul F-contraction, psum->sbuf copy + rope (DVE)."""
            b = h % 2
            m = 1 - ci // NJ           # 0..3 -> k (m=1), 4..7 -> q (m=0)
            j = ci % NJ
            if rr is None:
                pqt = pq_pool.tile([PP, SC], f32, tag="pq")
            else:
                pqt = psum_rr(rr, [PP, SC])
            for f in range(NF):
                nc.tensor.matmul(
                    pqt[:],
                    lhsT=wq_ap(b, f, m),
                    rhs=xt_ap(f, j),
                    start=(f == 0),
                    stop=(f == NF - 1),
                )
            qsb = vw_pool.tile([PP, SC], bf, tag="qsb")
            # DVE, not ACT: keeps the scalar engine pure-exp in the heads
            # phase (exp paces the PV lockstep; a copy in front of the next
            # iteration's first exp delays the whole chain).
            nc.vector.tensor_copy(qsb[:], pqt[:])
            # rotate_half = pair-swap of partitions (same permutation in every
            # 32-partition quadrant); the +-1 sign is folded into sint host-side
            qrot = vr_pool.tile([PP, SC], bf, tag="qrot")
            nc.vector.stream_shuffle(qrot[:], qsb[:], [i ^ 1 for i in range(32)])
            nc.vector.tensor_mul(qsb[:], qsb[:], cost_sb[:, ds(j * SC, SC)])
            nc.vector.tensor_mul(qrot[:], qrot[:], sint_sb[:, ds(j * SC, SC)])
            nc.vector.tensor_add(
                qk_sb[2 * (h % 2) + m][:, ds(j * SC, SC)], qsb[:], qrot[:])

        def psum_rr(i, shape):
            """Round-robin a [PP, SC] psum tile across the three psum pools."""
            pool, tag = ((pst_pool, "pst"), (py_pool, "py"), (pq_pool, "pq"))[i]
            return pool.tile(shape, f32, tag=tag, name=tag)

        def emit_proj_unit(t, ocp, rr0, rr1, on_act=False):
            """One [t-tile, 2*SC] unit of the output projection + one store.

            Two psum chains (adjacent SC columns), two casts to one bf16
            staging tile (split ACT/DVE so neither engine eats both), one
            2KB-run DMA."""
            ost = ost_pool.tile([PP, 2 * SC], bf, tag="ost")
            for half, rr in ((0, rr0), (1, rr1)):
                po = psum_rr(rr, [PP, SC])
                for cs in range(NCS):
                    nc.tensor.matmul(
                        po[:],
                        lhsT=y_sb[cs][:, ts(t, PP)],
                        rhs=wp_sb[cs][:, ds(ocp * 2 * SC + half * SC, SC)],
                        start=(cs == 0),
                        stop=(cs == NCS - 1),
                    )
                if (half == 0) == on_act:
                    nc.scalar.copy(ost[:, ds(half * SC, SC)], po[:])
                else:
                    nc.vector.tensor_copy(ost[:, ds(half * SC, SC)], po[:])
            # out-DMA alternates queues so neither HWDGE ring backs up
            if (t + ocp) % 2 == 0:
                nc.sync.dma_start(out_r[t][:, ds(ocp * 2 * SC, 2 * SC)], ost[:])
            else:
                nc.scalar.dma_start(out_r[t][:, ds(ocp * 2 * SC, 2 * SC)], ost[:])

        # ---------------- prologue: DMAs, head-0/1 qk, v GEMM ---------------
        # wq0 first (striped across all three queues, ~100 B/ns each), then
        # pure xt chunk-0 f-stripes so the four in-flight qk chains advance
        # at the aggregate arrival rate; rope-table halves and wq1 slices are
        # tucked in only where their consumers are about to need them.
        qs = (nc.sync, nc.scalar, nc.gpsimd)
        qs[0].dma_start(wq_sb[0][0][:], wqk_r[0][:, ds(0, 8 * PP)])
        qs[1].dma_start(wq_sb[0][1][:], wqk_r[0][:, ds(8 * PP, 8 * PP)])
        qs[2].dma_start(wq_sb[0][2][:], wqk_r[0][:, ds(16 * PP, 8 * PP)])
        qs[2].dma_start(wq_sb[0][3][:], wqk_r[0][:, ds(24 * PP, 8 * PP)])
        nc.scalar.dma_start(cost_sb[:, ds(0, C2)], cost_d.ap()[:, ds(0, C2)])
        nc.scalar.dma_start(sint_sb[:, ds(0, C2)], sint_d.ap()[:, ds(0, C2)])
        for f in range(NF):
            qs[f % 3].dma_start(xt_sb[f][0][:], xt_r[f][:, ds(0, SC)])
        for g in range(NF // 4):
            qs[g % 3].dma_start(wq_sb[1][g][:], wqk_r[1][:, ds(g * 8 * PP,
                                                               8 * PP)])
        for f in range(NF):
            qs[(f + 1) % 3].dma_start(xt_sb[f][1][:], xt_r[f][:, ds(SC, SC)])
        nc.scalar.dma_start(cost_sb[:, ds(C2, C2)], cost_d.ap()[:, ds(C2, C2)])
        nc.scalar.dma_start(sint_sb[:, ds(C2, C2)], sint_d.ap()[:, ds(C2, C2)])
        for i in range(NCS):           # wv: y-tile i <- packed cols [2i*CL]
            qs[i % 3].dma_start(y_sb[i][:], wv_r[:, ds(i * 2 * CL, 2 * CL)])
        for f in range(NF):
            qs[f % 3].dma_start(xt_sb[f][2][:], xt_r[f][:, ds(2 * SC, 2 * SC)])
        # (wp row-tiles 0-2 are preloaded later, at head 2: they are not
        # needed until head 7, and their 1.5MB would steal HBM bandwidth
        # from the xt/wv set that paces the DMA-starved prologue)

        def wv_ap(f, c):               # wv f-tile chunk c staged in y space
            return y_sb[f // 2][:, ds((f % 2) * CL + c * SC, SC)]

        def emit_v_tile(t):
            for c in range(CL // SC):
                ps = psum_rr(c % 2, [PP, SC])
                for f in range(NF):
                    nc.tensor.matmul(
                        ps[:],
                        lhsT=xt_tt(f, t),
                        rhs=wv_ap(f, c),
                        start=(f == 0),
                        stop=(f == NF - 1),
                    )
                nc.vector.tensor_copy(v_sb[t][:, ts(c, SC)], ps[:])

        # heads 0 AND 1 projected here: each arriving xt f-tile unlocks four
        # matmuls (k+q for two heads) instead of two, which is what keeps the
        # PE fed while the DMA engines stream the first megabytes in.  The
        # four chains stripe across all three psum pools (pq's 2-slot ring
        # alone would cap concurrency at two chains).
        for j in range(NJ):
            emit_qk_chunk(0, j, rr=2)           # k0 chunk j
            emit_qk_chunk(0, NJ + j, rr=0)      # q0 chunk j
            emit_qk_chunk(1, j, rr=1)           # k1 chunk j
            emit_qk_chunk(1, NJ + j, rr=2)      # q1 chunk j
        for t in range(NT):
            emit_v_tile(t)

        # ---------------- heads loop: attention + pipelined qk filler -------
        for h in range(HL):
            if h + 2 < HL:
                # wq for head h+2 into slot h%2 (read by qk_h, already done)
                b = (h + 2) % 2
                for g in range(NF // 4):
                    nc.gpsimd.dma_start(wq_sb[b][g][:],
                                        wqk_r[h + 2][:, ds(g * 8 * PP, 8 * PP)])
            if h == 2:
                for i in range(3):
                    nc.gpsimd.dma_start(wpre_sb[i][:], wp_r[i])
            if h == HL - 2:
                # xt's last readers (head-7 qk, emitted at h5) finish during
                # this head: close the pool now so the wp DMAs (WAR-gated on
                # those readers) land during h6, not h7
                xt_stack.close()
                wp_pool = octx.enter_context(tc.tile_pool(name="wp", bufs=1))
                wp_sb = wpre_sb + [
                    wp_pool.tile([PP, CO], bf, tag=f"wp{cs}", name=f"wp{cs}")
                    for cs in range(3, NCS)]
                # column-halves: the split chains and early proj units only
                # touch wp columns 0..1023, so land those first
                for cs in range(3, NCS):
                    qs[cs % 3].dma_start(
                        wp_sb[cs][:, ds(0, C2)], wp_r[cs][:, ds(0, C2)])
                for cs in range(3, NCS):
                    qs[cs % 3].dma_start(
                        wp_sb[cs][:, ds(C2, C2)], wp_r[cs][:, ds(C2, C2)])
                ost_pool = octx.enter_context(tc.tile_pool(name="ost", bufs=8))
                # split proj chains start HERE, at head 6: it has no qk filler
                # (head 7's was emitted at h5), so its PV tails are exp-paced
                # -- these chains (y0..y5 are done, wp0-2 preloaded) are the
                # only PE work whose inputs exist.  cs=6 lands at h7-c2=0,
                # cs=7 + store at h7's end.
                pend = []
                for oc in (0, 1):
                    po = pq_pool.tile([PP, SC], f32, tag="pq", name="pq")
                    for cs in range(NCS - 2):
                        nc.tensor.matmul(
                            po[:],
                            lhsT=y_sb[cs][:, ts(0, PP)],
                            rhs=wp_sb[cs][:, ds(oc * SC, SC)],
                            start=(cs == 0),
                            stop=False,
                        )
                    pend.append((oc, po))

            q_sb = qk_sb[2 * (h % 2)]
            k_sb = qk_sb[2 * (h % 2) + 1]
            for c2 in range(NC2):
                if h == HL - 1 and c2 == 0:
                    # continue the h6-parked split chains with y6's column
                    for oc, po in pend:
                        nc.tensor.matmul(
                            po[:],
                            lhsT=y_sb[NCS - 2][:, ts(0, PP)],
                            rhs=wp_sb[NCS - 2][:, ds(oc * SC, SC)],
                            start=False,
                            stop=False,
                        )
                # ST + exp + denominator running sum, kt-streamed
                es = []
                stot = None
                for kt in range(NT):
                    pst = pst_pool.tile([PP, C2], f32, tag="pst", name="pst")
                    for s in range(2):
                        nc.tensor.matmul(
                            pst[:, ts(s, SC)],
                            lhsT=k_sb[:, ts(kt, PP)],
                            rhs=q_sb[:, ds(c2 * C2 + s * SC, SC)],
                            start=True,
                            stop=True,
                        )
                    e = e_pool.tile([PP, C2], bf, tag="e", name="e")
                    nc.scalar.activation(
                        e[:], pst[:],
                        mybir.ActivationFunctionType.Exp,
                        bias=0.0, scale=scale,
                    )
                    es.append(e)
                    # running softmax-denominator sum: keeps the post-last-exp
                    # serial DVE tail to a single add
                    if kt == 1:
                        stot = vw_pool.tile([PP, C2], bf, tag="stot")
                        nc.vector.tensor_add(stot[:], es[0][:], es[1][:])
                    elif kt > 1:
                        nc.vector.tensor_add(stot[:], stot[:], es[kt][:])
                # PV: contract all k-tiles into y^T psum
                py_s = [py_pool.tile([PP, SC], f32, tag="py", name="py")
                        for s in range(2)]
                for kt in range(NT):
                    for s in range(2):
                        nc.tensor.matmul(
                            py_s[s][:],
                            lhsT=v_sb[kt][:, ts(h, PP)],
                            rhs=es[kt][:, ts(s, SC)],
                            start=(kt == 0),
                            stop=(kt == NT - 1),
                        )
                # denominator matmul + normalization: emitted BEFORE the
                # filler so the norm (which releases the py slots gating the
                # next iteration's PV) outranks the rope work
                pden = pst_pool.tile([PP, C2], f32, tag="pst", name="pst")
                for s in range(2):
                    nc.tensor.matmul(
                        pden[:, ts(s, SC)],
                        lhsT=ones_sb[:],
                        rhs=stot[:, ts(s, SC)],
                        start=True,
                        stop=True,
                    )
                for s in range(2):
                    inv = vw_pool.tile([PP, SC], f32, tag="inv", name="inv")
                    nc.vector.reciprocal_approx_fast(inv[:], pden[:, ts(s, SC)])
                    nc.vector.tensor_mul(
                        y_sb[h][:, ds(c2 * C2 + s * SC, SC)],
                        py_s[s][:], inv[:])
                # lower-priority PE filler: head h+2's qk GEMM, or early proj.
                # h+2 shares the qk slot head h is READING, so emission order
                # must put every slot-write after the last emitted ST that
                # reads that region: q cols 0:1024 are only read by c2=0's ST
                # (safe to emit in c2=0's filler), but k_sb is read by every
                # ST, so all k chunks (and q cols 1024:) go after c2=1's STs.
                # (The 16 matmuls of a chunk have no dependency on the qk
                # slot -- only the final DVE add does -- so the PE work flows
                # even while the add's WAR on head h's ST reads resolves.)
                if h + 2 < HL:
                    for ci in ((NJ, NJ + 1) if c2 == 0
                               else (NJ + 2, NJ + 3, 0, 1, 2, 3)):
                        emit_qk_chunk(h + 2, ci)
                elif h == HL - 1 and c2 == NC2 - 1:
                    ost = ost_pool.tile([PP, 2 * SC], bf, tag="ost")
                    for oc, po in pend:
                        nc.tensor.matmul(
                            po[:],
                            lhsT=y_sb[NCS - 1][:, ts(0, PP)],
                            rhs=wp_sb[NCS - 1][:, ds(oc * SC, SC)],
                            start=False,
                            stop=True,
                        )
                        nc.vector.tensor_copy(ost[:, ds(oc * SC, SC)], po[:])
                    nc.sync.dma_start(out_r[0][:, ds(0, 2 * SC)], ost[:])
                    for k, t in enumerate((1, 2, 3)):
                        emit_proj_unit(t, 0, 2, k % 2, on_act=(k % 2 == 1))

        # ---------------- tail: remaining output projection ------------------
        # units whose y7 columns landed with h7's first q-half (t < 8) go
        # first, on the uncontended pq slots, so they can fill h7-c2=1 stalls
        done = {(t, 0) for t in range(4)}
        early = [(t, ocp) for t in range(8) for ocp in range(NOC // 2)
                 if (t, ocp) not in done]
        late = [(t, ocp) for t in range(8, NT) for ocp in range(NOC // 2)]
        for i, (t, ocp) in enumerate(early + late):
            if i < 4:
                rr0, rr1 = 2, i % 2
            else:
                rr0, rr1 = i % 3, (i + 1) % 3
            emit_proj_unit(t, ocp, rr0, rr1, on_act=(i % 2 == 1 and i < 12))

    if compile_now:
        nc.compile()
    return nc


# ------------------------------------------------------------- host wrapper

def _percore_inputs(x, w_qkv, w_proj, core, HL=8):
    """Build the in_map for one core: batch b = core//2, head group g = core%2."""
    import ml_dtypes

    bf16 = ml_dtypes.bfloat16
    B, T, C = x.shape
    hd = HD
    NF = C // PP
    CL = HL * hd
    b, g = core // 2, core % 2

    cosT, sinT = _rope_tables(T)
    sign = np.where(np.arange(HD) % 2 == 0, -1.0, 1.0)[:, None]
    # per-head packed q|k weight columns, f-tile-blocked partition-major:
    # head m block = [f0: q128|k128, f1: q128|k128, ...] per partition row
    wqk = np.empty((C, HL * 2 * hd), np.float32)
    for m in range(HL):
        gm = g * HL + m
        wqk[:, m * 2 * hd: m * 2 * hd + hd] = w_qkv[:, gm * hd:(gm + 1) * hd]
        wqk[:, m * 2 * hd + hd:(m + 1) * 2 * hd] = \
            w_qkv[:, C + gm * hd: C + (gm + 1) * hd]
    wqk_p = np.concatenate(
        [_pmajor(wqk[:, m * 2 * hd:(m + 1) * 2 * hd], NF) for m in range(HL)],
        axis=1)
    vc0 = 2 * C + g * CL
    m = {
        "xt": _pmajor(np.ascontiguousarray(x[b].T), NF).astype(bf16),
        "wqk": wqk_p.astype(bf16),
        "wv": _pmajor(np.ascontiguousarray(w_qkv[:, vc0:vc0 + CL]), NF).astype(bf16),
        "wp": _pmajor(np.ascontiguousarray(w_proj[g * CL:(g + 1) * CL, :]),
                      HL).astype(bf16),
        "cost": cosT.astype(bf16),
        "sint": (sinT * sign).astype(bf16),
    }
    return m


def kernel(x, w_qkv, w_proj):
    from concourse.bass_utils import run_bass_kernel_spmd

    x = np.asarray(x, dtype=np.float32)
    w_qkv = np.asarray(w_qkv, dtype=np.float32)
    w_proj = np.asarray(w_proj, dtype=np.float32)
    B, T, C = x.shape
    HL = N_HEAD // (N_CORES // B)

    key = (T, C, HL, C)
    if key not in _NC_CACHE:
        _NC_CACHE[key] = build_nc(T, C, HL, C)
    nc = _NC_CACHE[key]

    in_maps = [_percore_inputs(x, w_qkv, w_proj, c, HL) for c in range(N_CORES)]
    trace = bool(int(os.environ.get("KERNEL_TRACE", "0")))
    res = run_bass_kernel_spmd(
        nc, in_maps, core_ids=list(range(N_CORES)), trace=trace)
    if trace:
        global LAST_EXEC_TIME_NS, LAST_RESULT
        LAST_EXEC_TIME_NS = res.exec_time_ns
        LAST_RESULT = res

    out = np.empty((B, T, C), np.float32)
    for b in range(B):
        out[b] = (res.results[2 * b]["out"].astype(np.float32)
                  + res.results[2 * b + 1]["out"].astype(np.float32))
    return out


LAST_EXEC_TIME_NS = None
LAST_RESULT = None
